# revision 4
# baseline (speedup 1.0000x reference)
"""Trainium2 Bass kernel for nn_MessagePassing_42588895707817.

out = (h @ W.T + b) @ norm_graph,  norm_graph = graph / clip(rowsum(graph), EPS)

Math folding: out = h @ C + d  with  C = W.T @ norm_graph  (128x128),
d = b @ norm_graph (zeros here; handled exactly on host if nonzero).

Device does ONLY the big streaming matmul, in bf16 (rel-err budget 2e-2
dwarfs bf16's ~0.5%): per core 32768 tokens x 128 feat.  The host
pre-transposes h to [f, tok] blocks so the PE needs no on-chip
transpose: matmul(out_T[g, tok], lhsT=C[f, g], rhs=hT[f, tok]).  HBM
traffic per core is 8.4 MB in + 8.4 MB out -> ~40 us of DMA at the
observed ~425 GB/s active rate, plus ~8 us fixed NEFF/Tile preamble.

DMA stays coarse (1 MB chunks -> dense queues); only the FIRST chunk's
load and the LAST chunk's store are split so the pipeline starts ~2 us
earlier and drains ~1.5 us faster.

Sharding: data-parallel on batch B=32 across 8 cores (4 batches/core).
"""

import sys

if "/opt/trn_rl_repo" not in sys.path:
    sys.path.insert(0, "/opt/trn_rl_repo")

from contextlib import ExitStack

import ml_dtypes
import numpy as np

B, T, FDIM, HID = 32, 8192, 128, 128
EPS = 1e-10
NCORES = 8
B_LOC = B // NCORES
NTOK = B_LOC * T  # 32768 tokens per core

P = 128  # partitions
MMN = 512  # matmul free dim (one PSUM bank of fp32)
GRP = 2  # PSUM banks (= matmuls) per engine copy
CH = 4096  # tokens per chunk
NCHUNK = NTOK // CH


def build_program(ntok=NTOK, ch=CH):
    import concourse.bacc as bacc
    import concourse.tile as tile
    from concourse import mybir

    f32 = mybir.dt.float32
    bf16 = mybir.dt.bfloat16
    nchunk = ntok // ch
    nmm = ch // MMN

    nc = bacc.Bacc("TRN2", debug=False, target_bir_lowering=False)

    ht_d = nc.dram_tensor("ht", [nchunk, P, ch], bf16, kind="ExternalInput")
    c_d = nc.dram_tensor("C", [P, P], bf16, kind="ExternalInput")
    out_d = nc.dram_tensor("out", [nchunk, P, ch], bf16, kind="ExternalOutput")

    with tile.TileContext(nc) as tc, ExitStack() as ctx:
        singles = ctx.enter_context(tc.tile_pool(name="singles", bufs=1))
        ld = ctx.enter_context(tc.tile_pool(name="ld", bufs=6))
        st = ctx.enter_context(tc.tile_pool(name="st", bufs=4))
        ps = ctx.enter_context(tc.tile_pool(name="ps", bufs=4, space="PSUM"))

        c_raw = singles.tile([P, P], bf16)
        nc.sync.dma_start(out=c_raw, in_=c_d[:])
        # Stage C through DVE so matmuls only ever wait on one sem each.
        c_s = singles.tile([P, P], bf16)
        nc.vector.tensor_copy(c_s, c_raw)

        k = 0
        for c in range(nchunk):
            in_t = ld.tile([P, ch], bf16)
            if c == 0:
                # split: first matmul only waits on the leading 512 tokens
                nc.sync.dma_start(out=in_t[:, :MMN], in_=ht_d[c][:, :MMN])
                nc.sync.dma_start(out=in_t[:, MMN:2 * MMN],
                                  in_=ht_d[c][:, MMN:2 * MMN])
                nc.sync.dma_start(out=in_t[:, 2 * MMN:], in_=ht_d[c][:, 2 * MMN:])
            else:
                nc.sync.dma_start(out=in_t, in_=ht_d[c])
            out_t = st.tile([P, ch], bf16)
            for g0 in range(0, nmm, GRP):
                gn = min(GRP, nmm - g0)
                o_ps = ps.tile([P, GRP, MMN], f32)
                for j in range(gn):
                    nc.tensor.matmul(o_ps[:, j, :], lhsT=c_s,
                                     rhs=in_t[:, (g0 + j) * MMN:(g0 + j + 1) * MMN],
                                     start=True, stop=True)
                src = o_ps[:, :gn, :]
                dst = out_t[:, g0 * MMN:(g0 + gn) * MMN]
                if k % 2 == 0:
                    nc.vector.tensor_copy(dst, src)
                else:
                    nc.scalar.copy(dst, src)
                k += 1
            if c == nchunk - 1:
                # split: final store drains in quarters so the tail after the
                # last copy is only 256 KB
                q = ch // 4
                for i in range(4):
                    nc.scalar.dma_start(out=out_d[c][:, i * q:(i + 1) * q],
                                        in_=out_t[:, i * q:(i + 1) * q])
            else:
                nc.scalar.dma_start(out=out_d[c], in_=out_t)

    nc.compile()
    return nc


def _fold_constants(graph, W, b):
    """C = W.T @ norm_graph (bf16), d = b @ norm_graph (fp32, exact path)."""
    g = np.asarray(graph, dtype=np.float64)
    deg = np.clip(g.sum(axis=1, keepdims=True), EPS, None)
    norm = np.where(deg > EPS, g / deg, 0.0)
    C = (np.asarray(W, dtype=np.float64).T @ norm).astype(ml_dtypes.bfloat16)
    d = (np.asarray(b, dtype=np.float64) @ norm).astype(np.float32)
    return C, d


def make_in_maps(h, graph, W, b, ch=CH):
    nchunk = NTOK // ch
    C, _ = _fold_constants(graph, W, b)
    hb = np.asarray(h, dtype=np.float32).reshape(NCORES, NTOK, FDIM)
    hb = hb.astype(ml_dtypes.bfloat16)
    return [
        {
            "ht": np.ascontiguousarray(
                hb[i].reshape(nchunk, ch, FDIM).transpose(0, 2, 1)
            ),
            "C": C,
        }
        for i in range(NCORES)
    ]


def unpack_outputs(res, b_d, ch=CH):
    nchunk = NTOK // ch
    outs = []
    for i in range(NCORES):
        r = res.results[i]["out"].reshape(nchunk, HID, ch)
        o = r.transpose(0, 2, 1).reshape(B_LOC, T, HID).astype(np.float32)
        outs.append(o)
    out = np.concatenate(outs, axis=0)
    if b_d is not None:
        out = out + b_d[None, None, :]
    return out


_LDW_PATCHED = False


def _enable_ldw_opt(bass_utils):
    """Compile walrus with --enable-ldw-opt=true: lets the PE hide LDWEIGHTS
    behind in-flight matmuls."""
    global _LDW_PATCHED
    if _LDW_PATCHED:
        return
    _LDW_PATCHED = True
    orig = bass_utils.run_command

    def patched(argv, **kw):
        argv = [a.replace("--enable-ldw-opt=false", "--enable-ldw-opt=true")
                if isinstance(a, str) else a for a in argv]
        return orig(argv, **kw)

    bass_utils.run_command = patched


def kernel(h, graph, W, b):
    from concourse import bass_utils

    _enable_ldw_opt(bass_utils)
    nc = build_program()
    in_maps = make_in_maps(h, graph, W, b)
    res = bass_utils.run_bass_kernel_spmd(nc, in_maps, list(range(NCORES)))
    b_np = np.asarray(b, dtype=np.float64)
    d = _fold_constants(graph, W, b)[1] if np.any(b_np) else None
    return unpack_outputs(res, d)


# revision 8
# speedup vs baseline: 1.0376x; 1.0376x over previous
"""Trainium2 Bass kernel for nn_MessagePassing_42588895707817.

out = (h @ W.T + b) @ norm_graph,  norm_graph = graph / clip(rowsum(graph), EPS)

Math folding: out = h @ C + d  with  C = W.T @ norm_graph  (128x128),
d = b @ norm_graph (zeros here; handled exactly on host if nonzero).

Device does ONLY the big streaming matmul, in bf16 (rel-err budget 2e-2
dwarfs bf16's ~0.5%): per core 32768 tokens x 128 feat.  The host
pre-transposes h to [f, tok] blocks so the PE needs no on-chip
transpose: matmul(out_T[g, tok], lhsT=C[f, g], rhs=hT[f, tok]).  HBM
traffic per core is 8.4 MB in + 8.4 MB out -> ~40 us of DMA at the
observed ~425 GB/s active rate, plus ~8 us fixed NEFF/Tile preamble.

DMA stays coarse (1 MB chunks -> dense queues); only the FIRST chunk's
load and the LAST chunk's store are split so the pipeline starts ~2 us
earlier and drains ~1.5 us faster.

Sharding: data-parallel on batch B=32 across 8 cores (4 batches/core).
"""

import sys

if "/opt/trn_rl_repo" not in sys.path:
    sys.path.insert(0, "/opt/trn_rl_repo")

from contextlib import ExitStack

import ml_dtypes
import numpy as np

B, T, FDIM, HID = 32, 8192, 128, 128
EPS = 1e-10
NCORES = 8
B_LOC = B // NCORES
NTOK = B_LOC * T  # 32768 tokens per core

P = 128  # partitions
MMN = 512  # matmul free dim (one PSUM bank of fp32)
GRP = 2  # PSUM banks (= matmuls) per engine copy
CH = 4096  # tokens per chunk
NCHUNK = NTOK // CH


def build_program(ntok=NTOK, ch=CH):
    import concourse.bacc as bacc
    import concourse.tile as tile
    from concourse import mybir

    f32 = mybir.dt.float32
    bf16 = mybir.dt.bfloat16
    nchunk = ntok // ch
    nmm = ch // MMN

    nc = bacc.Bacc("TRN2", debug=False, target_bir_lowering=False)

    ht_d = nc.dram_tensor("ht", [nchunk, P, ch], bf16, kind="ExternalInput")
    c_d = nc.dram_tensor("C", [P, P], bf16, kind="ExternalInput")
    out_d = nc.dram_tensor("out", [nchunk, P, ch], bf16, kind="ExternalOutput")

    with tile.TileContext(nc) as tc, ExitStack() as ctx:
        singles = ctx.enter_context(tc.tile_pool(name="singles", bufs=1))
        # ld bufs == nchunk: the whole 8.4 MB input shard is SBUF-resident, so
        # loads never wait on compute and the input queue streams at line rate;
        # the PE is then continuously fed (load cadence < compute cadence) and
        # warms its HAM clock gate naturally.
        ld = ctx.enter_context(tc.tile_pool(name="ld", bufs=nchunk))
        st = ctx.enter_context(tc.tile_pool(name="st", bufs=4))
        ps = ctx.enter_context(tc.tile_pool(name="ps", bufs=4, space="PSUM"))

        c_raw = singles.tile([P, P], bf16)
        nc.sync.dma_start(out=c_raw, in_=c_d[:])
        # Stage C through DVE so matmuls only ever wait on one sem each.
        c_s = singles.tile([P, P], bf16)
        nc.vector.tensor_copy(c_s, c_raw)

        k = 0
        for c in range(nchunk):
            in_t = ld.tile([P, ch], bf16)
            if c == 0:
                # split: first matmul only waits on the leading 512 tokens
                nc.sync.dma_start(out=in_t[:, :MMN], in_=ht_d[c][:, :MMN])
                nc.sync.dma_start(out=in_t[:, MMN:2 * MMN],
                                  in_=ht_d[c][:, MMN:2 * MMN])
                nc.sync.dma_start(out=in_t[:, 2 * MMN:], in_=ht_d[c][:, 2 * MMN:])
            else:
                nc.sync.dma_start(out=in_t, in_=ht_d[c])
            out_t = st.tile([P, ch], bf16)
            for g0 in range(0, nmm, GRP):
                gn = min(GRP, nmm - g0)
                o_ps = ps.tile([P, GRP, MMN], f32)
                for j in range(gn):
                    nc.tensor.matmul(o_ps[:, j, :], lhsT=c_s,
                                     rhs=in_t[:, (g0 + j) * MMN:(g0 + j + 1) * MMN],
                                     start=True, stop=True)
                src = o_ps[:, :gn, :]
                dst = out_t[:, g0 * MMN:(g0 + gn) * MMN]
                if k % 2 == 0:
                    nc.vector.tensor_copy(dst, src)
                else:
                    nc.scalar.copy(dst, src)
                k += 1
            if c == nchunk - 1:
                # split: final store drains in quarters so the tail after the
                # last copy is only 256 KB
                q = ch // 4
                for i in range(4):
                    nc.scalar.dma_start(out=out_d[c][:, i * q:(i + 1) * q],
                                        in_=out_t[:, i * q:(i + 1) * q])
            else:
                nc.scalar.dma_start(out=out_d[c], in_=out_t)

    nc.compile()
    return nc


def _fold_constants(graph, W, b):
    """C = W.T @ norm_graph (bf16), d = b @ norm_graph (fp32, exact path)."""
    g = np.asarray(graph, dtype=np.float64)
    deg = np.clip(g.sum(axis=1, keepdims=True), EPS, None)
    norm = np.where(deg > EPS, g / deg, 0.0)
    C = (np.asarray(W, dtype=np.float64).T @ norm).astype(ml_dtypes.bfloat16)
    d = (np.asarray(b, dtype=np.float64) @ norm).astype(np.float32)
    return C, d


def make_in_maps(h, graph, W, b, ch=CH):
    nchunk = NTOK // ch
    C, _ = _fold_constants(graph, W, b)
    hb = np.asarray(h, dtype=np.float32).reshape(NCORES, NTOK, FDIM)
    hb = hb.astype(ml_dtypes.bfloat16)
    return [
        {
            "ht": np.ascontiguousarray(
                hb[i].reshape(nchunk, ch, FDIM).transpose(0, 2, 1)
            ),
            "C": C,
        }
        for i in range(NCORES)
    ]


def unpack_outputs(res, b_d, ch=CH):
    nchunk = NTOK // ch
    outs = []
    for i in range(NCORES):
        r = res.results[i]["out"].reshape(nchunk, HID, ch)
        o = r.transpose(0, 2, 1).reshape(B_LOC, T, HID).astype(np.float32)
        outs.append(o)
    out = np.concatenate(outs, axis=0)
    if b_d is not None:
        out = out + b_d[None, None, :]
    return out


_LDW_PATCHED = False


def _enable_ldw_opt(bass_utils):
    """Compile walrus with --enable-ldw-opt=true: lets the PE hide LDWEIGHTS
    behind in-flight matmuls."""
    global _LDW_PATCHED
    if _LDW_PATCHED:
        return
    _LDW_PATCHED = True
    orig = bass_utils.run_command

    def patched(argv, **kw):
        argv = [a.replace("--enable-ldw-opt=false", "--enable-ldw-opt=true")
                if isinstance(a, str) else a for a in argv]
        return orig(argv, **kw)

    bass_utils.run_command = patched


def kernel(h, graph, W, b):
    from concourse import bass_utils

    _enable_ldw_opt(bass_utils)
    nc = build_program()
    in_maps = make_in_maps(h, graph, W, b)
    res = bass_utils.run_bass_kernel_spmd(nc, in_maps, list(range(NCORES)))
    b_np = np.asarray(b, dtype=np.float64)
    d = _fold_constants(graph, W, b)[1] if np.any(b_np) else None
    return unpack_outputs(res, d)


# revision 12
# speedup vs baseline: 1.0565x; 1.0183x over previous
"""Trainium2 Bass kernel for nn_MessagePassing_42588895707817.

out = (h @ W.T + b) @ norm_graph,  norm_graph = graph / clip(rowsum(graph), EPS)

Math folding: out = h @ C + d  with  C = W.T @ norm_graph  (128x128),
d = b @ norm_graph (zeros here; handled exactly on host if nonzero).

Device does ONLY the big streaming matmul, in bf16 (rel-err budget 2e-2
dwarfs bf16's ~0.5%): per core 32768 tokens x 128 feat.  The host
pre-transposes h to [f, tok] blocks so the PE needs no on-chip
transpose: matmul(out_T[g, tok], lhsT=C[f, g], rhs=hT[f, tok]).  HBM
traffic per core is 8.4 MB in + 8.4 MB out -> ~40 us of DMA at the
observed ~425 GB/s active rate, plus ~8 us fixed NEFF/Tile preamble.

DMA stays coarse (1 MB chunks -> dense queues); only the FIRST chunk's
load and the LAST chunk's store are split so the pipeline starts ~2 us
earlier and drains ~1.5 us faster.

Sharding: data-parallel on batch B=32 across 8 cores (4 batches/core).
"""

import sys

if "/opt/trn_rl_repo" not in sys.path:
    sys.path.insert(0, "/opt/trn_rl_repo")

from contextlib import ExitStack

import ml_dtypes
import numpy as np

B, T, FDIM, HID = 32, 8192, 128, 128
EPS = 1e-10
NCORES = 8
B_LOC = B // NCORES
NTOK = B_LOC * T  # 32768 tokens per core

P = 128  # partitions
MMN = 512  # matmul free dim (one PSUM bank of fp32)
GRP = 2  # PSUM banks (= matmuls) per engine copy
CH = 8192  # tokens per chunk (2 MB DMA transfers; fewer, denser queues)
NCHUNK = NTOK // CH
STSPLIT = 1  # store pieces per non-final chunk (final chunk always splits 4x)


def build_program(ntok=NTOK, ch=CH):
    import concourse.bacc as bacc
    import concourse.tile as tile
    from concourse import mybir

    f32 = mybir.dt.float32
    bf16 = mybir.dt.bfloat16
    nchunk = ntok // ch
    nmm = ch // MMN

    nc = bacc.Bacc("TRN2", debug=False, target_bir_lowering=False)

    ht_d = nc.dram_tensor("ht", [nchunk, P, ch], bf16, kind="ExternalInput")
    c_d = nc.dram_tensor("C", [P, P], bf16, kind="ExternalInput")
    out_d = nc.dram_tensor("out", [nchunk, P, ch], bf16, kind="ExternalOutput")

    with tile.TileContext(nc) as tc, ExitStack() as ctx:
        singles = ctx.enter_context(tc.tile_pool(name="singles", bufs=1))
        # ld bufs == nchunk: the whole 8.4 MB input shard is SBUF-resident, so
        # loads never wait on compute and the input queue streams at line rate;
        # the PE is then continuously fed (load cadence < compute cadence) and
        # warms its HAM clock gate naturally.
        ld = ctx.enter_context(tc.tile_pool(name="ld", bufs=nchunk))
        st = ctx.enter_context(tc.tile_pool(name="st", bufs=4))
        ps = ctx.enter_context(tc.tile_pool(name="ps", bufs=4, space="PSUM"))

        c_raw = singles.tile([P, P], bf16)
        nc.sync.dma_start(out=c_raw, in_=c_d[:])
        # Stage C through DVE so matmuls only ever wait on one sem each.
        c_s = singles.tile([P, P], bf16)
        nc.vector.tensor_copy(c_s, c_raw)

        k = 0
        for c in range(nchunk):
            in_t = ld.tile([P, ch], bf16)
            if c == 0:
                # split: first matmul only waits on the leading 512 tokens
                nc.sync.dma_start(out=in_t[:, :MMN], in_=ht_d[c][:, :MMN])
                nc.sync.dma_start(out=in_t[:, MMN:2 * MMN],
                                  in_=ht_d[c][:, MMN:2 * MMN])
                nc.sync.dma_start(out=in_t[:, 2 * MMN:], in_=ht_d[c][:, 2 * MMN:])
            else:
                nc.sync.dma_start(out=in_t, in_=ht_d[c])
            out_t = st.tile([P, ch], bf16)
            for g0 in range(0, nmm, GRP):
                gn = min(GRP, nmm - g0)
                o_ps = ps.tile([P, GRP, MMN], f32)
                for j in range(gn):
                    nc.tensor.matmul(o_ps[:, j, :], lhsT=c_s,
                                     rhs=in_t[:, (g0 + j) * MMN:(g0 + j + 1) * MMN],
                                     start=True, stop=True)
                src = o_ps[:, :gn, :]
                dst = out_t[:, g0 * MMN:(g0 + gn) * MMN]
                if k % 2 == 0:
                    nc.vector.tensor_copy(dst, src)
                else:
                    nc.scalar.copy(dst, src)
                k += 1
            nst = 4 if c == nchunk - 1 else STSPLIT
            q = ch // nst
            for i in range(nst):
                nc.scalar.dma_start(out=out_d[c][:, i * q:(i + 1) * q],
                                    in_=out_t[:, i * q:(i + 1) * q])

    nc.compile()
    return nc


def _fold_constants(graph, W, b):
    """C = W.T @ norm_graph (bf16), d = b @ norm_graph (fp32, exact path)."""
    g = np.asarray(graph, dtype=np.float64)
    deg = np.clip(g.sum(axis=1, keepdims=True), EPS, None)
    norm = np.where(deg > EPS, g / deg, 0.0)
    C = (np.asarray(W, dtype=np.float64).T @ norm).astype(ml_dtypes.bfloat16)
    d = (np.asarray(b, dtype=np.float64) @ norm).astype(np.float32)
    return C, d


def make_in_maps(h, graph, W, b, ch=CH):
    nchunk = NTOK // ch
    C, _ = _fold_constants(graph, W, b)
    hb = np.asarray(h, dtype=np.float32).reshape(NCORES, NTOK, FDIM)
    hb = hb.astype(ml_dtypes.bfloat16)
    return [
        {
            "ht": np.ascontiguousarray(
                hb[i].reshape(nchunk, ch, FDIM).transpose(0, 2, 1)
            ),
            "C": C,
        }
        for i in range(NCORES)
    ]


def unpack_outputs(res, b_d, ch=CH):
    nchunk = NTOK // ch
    outs = []
    for i in range(NCORES):
        r = res.results[i]["out"].reshape(nchunk, HID, ch)
        o = r.transpose(0, 2, 1).reshape(B_LOC, T, HID).astype(np.float32)
        outs.append(o)
    out = np.concatenate(outs, axis=0)
    if b_d is not None:
        out = out + b_d[None, None, :]
    return out


def kernel(h, graph, W, b):
    from concourse import bass_utils

    nc = build_program()
    in_maps = make_in_maps(h, graph, W, b)
    res = bass_utils.run_bass_kernel_spmd(nc, in_maps, list(range(NCORES)))
    b_np = np.asarray(b, dtype=np.float64)
    d = _fold_constants(graph, W, b)[1] if np.any(b_np) else None
    return unpack_outputs(res, d)
